# revision 18
# baseline (speedup 1.0000x reference)
"""Multi-head self-attention (QK^T -> softmax -> ctx -> linear) on 8 TRN2 cores.

Sharding: each core owns one (batch, query-block) shard: batch = core//4,
queries [qlo, qlo+512) with qlo = (core%4)*512. Attention needs all keys of
the core's batch, so keys are replicated per batch; no collectives needed.

Per core (head h, its 512 queries q, all 2048 keys k), all operands bf16:
  S_T[k, q]  = sum_d x[k, hd+d] * x[q, hd+d]            (PE, psum f32)
  P_T[k, q]  = exp(0.125 * S_T[k, q])                   (ACT, psum->sbuf bf16)
  ctx[q, m]  = sum_k P_T[k, q] * xa[k, m]               (PE; P parked as the
               stationary operand so only the 65-wide xa side streams;
               m = 64 dims + ones column -> softmax denominator at m=64)
  chunk[q,i] = ctx[q, d] / ctx[q, 64]                   (DVE tensor_scalar,
               per-partition recip scalar; -> bf16)
  chunkT     = transpose(chunk)                         (DMA xbar transpose)
  out[q, o]  = sum_i chunkT[i, q] * W[o, i] + b[o]      (PE + DVE adds)

The PE cost model charges only streamed output columns (stationary loads are
free), so parking P halves ctx cost vs streaming it; everything else is
orientation-chosen to keep output partitions full.
"""

import sys

for _p in ("/opt/trn_rl_repo", "/root/.axon_site/_ro/trn_rl_repo"):
    if _p not in sys.path:
        sys.path.append(_p)

import numpy as np

import concourse.bacc as bacc
import concourse.bass as bass
import concourse.mybir as mybir
import concourse.tile as tile

F32 = mybir.dt.float32
BF16 = mybir.dt.bfloat16

B, L, H, NH, DH = 2, 2048, 1024, 16, 64
NCORES = 8
QB = 512
KBLKS = L // 128          # 16 key blocks
NPAIR = NH // 2           # 8 head pairs
AUG = DH + 1              # 64 dims + ones column
HC = H // 128             # 8 hidden chunks (one per pair)
OBW = 512                 # proj output column block
SCALE = float(1.0 / np.sqrt(DH))
QSUB = QB // 128          # 4 query subtiles


def build_nc():
    nc = bacc.Bacc("TRN2")
    xt = nc.declare_dram_parameter("xt", [NPAIR, 128, L], BF16, isOutput=False)
    xq = nc.declare_dram_parameter("xq", [NPAIR, 128, QB], BF16, isOutput=False)
    xa = nc.declare_dram_parameter("xa", [NPAIR, 128, 2 * KBLKS * AUG], BF16, isOutput=False)
    wt = nc.declare_dram_parameter("wt", [128, HC * H], BF16, isOutput=False)
    bias = nc.declare_dram_parameter("bias", [1, H], BF16, isOutput=False)
    ones = nc.declare_dram_parameter("ones", [1, 128], BF16, isOutput=False)
    ident = nc.declare_dram_parameter("ident", [128, 128], BF16, isOutput=False)
    out = nc.declare_dram_parameter("out", [QB, H], F32, isOutput=True)

    NSTEP = NPAIR * KBLKS
    SKEW = 2
    # scheduling floor per global step (ns): keeps the tile scheduler from
    # hoisting proj work into much earlier PE positions, where an unmet
    # transpose dep would stall the PE counter that gates the exp stream.
    TSTEP_NS = 1000.0
    TBASE_NS = 3000.0

    def floor_ms(gs):
        return (TBASE_NS + gs * TSTEP_NS) / 1e6

    with tile.TileContext(nc) as tc:
        with (
            tc.tile_pool(name="xt", bufs=2) as xt_pool,
            tc.tile_pool(name="xq", bufs=2) as xq_pool,
            tc.tile_pool(name="xa", bufs=2) as xa_pool,
            tc.tile_pool(name="p", bufs=4) as p_pool,
            tc.tile_pool(name="cq", bufs=8) as cq_pool,
            tc.tile_pool(name="rc", bufs=4) as rc_pool,
            tc.tile_pool(name="consts", bufs=1) as consts,
            tc.tile_pool(name="spsum", bufs=2, space="PSUM") as s_psum,
            tc.tile_pool(name="ctxpsum", bufs=3, space="PSUM") as ctx_psum,
            tc.tile_pool(name="prjpsum", bufs=1, space="PSUM") as prj_psum,
        ):
            ones_t = consts.tile([1, 128], BF16)
            ident_t = consts.tile([128, 128], BF16)
            bias_t = consts.tile([1, H], BF16)
            bias_bc = consts.tile([128, H], F32)

            wt_ts = [
                consts.tile([128, H], BF16, tag=f"wt{c}", name=f"wt{c}")
                for c in range(HC)
            ]
            chunks = [
                consts.tile([128, QB], BF16, tag=f"ch{c}", name=f"ch{c}")
                for c in range(HC)
            ]
            acc = [
                consts.tile([128, H], F32, tag=f"acc{q}", name=f"acc{q}")
                for q in range(QSUB)
            ]

            def pair_dmas(pr, split_xt=False):
                xt_t = xt_pool.tile([128, L], BF16, tag="xt", name=f"xt{pr}")
                xq_t = xq_pool.tile([128, QB], BF16, tag="xq", name=f"xq{pr}")
                nc.sync.dma_start(xq_t[:], xq[pr])
                if split_xt:
                    # first key quarter lands fast so scores can start early
                    nc.sync.dma_start(xt_t[:, 0:512], xt[pr][:, 0:512])
                    nc.sync.dma_start(xt_t[:, 512:L], xt[pr][:, 512:L])
                else:
                    nc.sync.dma_start(xt_t[:], xt[pr])
                xa_t = xa_pool.tile([128, 2 * KBLKS * AUG], BF16, tag="xa", name=f"xa{pr}")
                nc.sync.dma_start(xa_t[:], xa[pr])
                return [xt_t, xq_t, xa_t, None, None]

            def emit_proj_group(clist, qs, ob, pool, tag, eng=None):
                obsl = slice(ob * OBW, (ob + 1) * OBW)
                qsl = slice(qs * 128, (qs + 1) * 128)
                cp = pool.tile([128, 2 * QB] if tag == "s" else [128, OBW],
                               F32, tag=tag, name=f"cp{clist[0]}_{qs}_{ob}")
                for i, c in enumerate(clist):
                    nc.tensor.matmul(
                        cp[:, 0:OBW], chunks[c][:, qsl], wt_ts[c][:, obsl],
                        start=(i == 0), stop=(i == len(clist) - 1),
                    )
                prev = bias_bc if clist[0] == 0 else acc[qs]
                (eng or nc.vector).tensor_add(
                    acc[qs][:, obsl], cp[:, 0:OBW], prev[:, obsl]
                )

            def emit_drain(prp, ctx_a, ctx_b):
                # normalize pair prp's ctx accumulators and transpose into
                # the proj chunk tile; for the final pair, pipeline the
                # remaining projection groups and output DMAs per q-subtile
                last = prp == NPAIR - 1
                rc_a = rc_pool.tile([128, QSUB], F32, tag="rc", name=f"rca{prp}")
                rc_b = rc_pool.tile([128, QSUB], F32, tag="rc", name=f"rcb{prp}")
                with nc.allow_low_precision(reason="softmax denominator recip"):
                    nc.vector.reciprocal(rc_a[:], ctx_a[:, DH : QSUB * AUG : AUG])
                    nc.vector.reciprocal(rc_b[:], ctx_b[:, DH : QSUB * AUG : AUG])
                for qs in range(QSUB):
                    cq = cq_pool.tile([128, 128], BF16, tag="cq", name=f"cq{prp}_{qs}")
                    nc.vector.tensor_scalar_mul(
                        cq[:, 0:DH], ctx_a[:, qs * AUG : qs * AUG + DH],
                        rc_a[:, qs : qs + 1],
                    )
                    nc.vector.tensor_scalar_mul(
                        cq[:, DH : 2 * DH], ctx_b[:, qs * AUG : qs * AUG + DH],
                        rc_b[:, qs : qs + 1],
                    )
                    if not last:
                        nc.sync.dma_start_transpose(
                            chunks[prp][:, qs * 128 : (qs + 1) * 128], cq[:, :]
                        )
                        continue
                    # final pair: PE-transpose (HWDGE is slow to drain) and a
                    # single fused (5,6,7) projection + add + out DMA per qs
                    tp = s_psum.tile([128, 128], BF16, tag="s", name=f"tp{qs}")
                    nc.tensor.transpose(tp[:], cq[:, :], ident_t[:])
                    qsl = slice(qs * 128, (qs + 1) * 128)
                    nc.scalar.copy(chunks[prp][:, qsl], tp[:])
                    cp = s_psum.tile([128, 2 * QB], F32, tag="s", name=f"tcp{qs}")
                    for ob in range(2):
                        obsl = slice(ob * OBW, (ob + 1) * OBW)
                        for i, c in enumerate((5, 6, 7)):
                            nc.tensor.matmul(
                                cp[:, obsl], chunks[c][:, qsl], wt_ts[c][:, obsl],
                                start=(i == 0), stop=(i == 2),
                            )
                    nc.vector.tensor_add(acc[qs][:], cp[:], acc[qs][:])
                    nc.sync.dma_start(out[qsl, :], acc[qs][:])

            # proj emission plan: step -> (chunk list, qs, ob)
            # chunk-pairs (0,1)@pairs2-3, (2,3)@pairs4-5; chunk 4 singles@pair6,
            # (5,6)@pair7, chunk 7 + leftovers in the tail.
            # window steps start ~4 kbs after the newer chunk's drain so the
            # first group never waits on an in-flight transpose
            plan = {}
            for hp, base_pr in ((0, 2), (1, 4)):
                cl = [2 * hp, 2 * hp + 1]
                steps = [base_pr * KBLKS + k for k in (6, 8, 10, 12, 14)] + [
                    (base_pr + 1) * KBLKS + k for k in (4, 8, 12)
                ]
                for gidx, st in enumerate(steps):
                    qs, ob = divmod(gidx, 2)
                    plan[st] = (cl, qs, ob)
            for i, kbw in enumerate((3, 5, 7, 9, 11, 13, 15)):
                plan[6 * KBLKS + kbw] = ([4], i // 2, i % 2)
            plan[7 * KBLKS + 1] = ([4], 3, 1)

            tiles = {}
            pipe = []
            for gs in range(NSTEP + SKEW):
                cur = None
                if gs < NSTEP:
                    pr, kb = divmod(gs, KBLKS)
                    if pr == 0 and kb == 0:
                        tiles[0] = pair_dmas(0, split_xt=True)
                        nc.sync.dma_start(wt_ts[0][:], wt[:, 0:H])
                        nc.sync.dma_start(ones_t[:], ones[:])
                        nc.sync.dma_start(bias_t[:], bias[:])
                        nc.sync.dma_start(ident_t[:], ident[:])
                    if pr == 0 and kb == 6:
                        # bias broadcast across partitions via K=1 ones matmul
                        # (uses the proj psum bank, idle until pair 2, so the
                        # late-landing bias DMA never stalls the score slots)
                        for ob in range(H // OBW):
                            obsl = slice(ob * OBW, (ob + 1) * OBW)
                            bps = prj_psum.tile([128, OBW], F32, tag="prj")
                            nc.tensor.matmul(
                                bps[:], ones_t[0:1, :], bias_t[0:1, obsl],
                                start=True, stop=True,
                            )
                            nc.vector.tensor_copy(bias_bc[:, obsl], bps[:])
                    if kb == 2 and pr + 1 < NPAIR:
                        # floor keeps prefetches behind the previous
                        # drain's transposes in the HWDGE queue
                        with tc.tile_wait_until(floor_ms((pr + 1) * KBLKS - 6)):
                            tiles[pr + 1] = pair_dmas(pr + 1)
                    if kb == 4 and pr + 1 < NPAIR:
                        with tc.tile_wait_until(floor_ms(min((pr + 3) * KBLKS - 10, NSTEP))):
                            nc.sync.dma_start(
                                wt_ts[pr + 1][:], wt[:, (pr + 1) * H : (pr + 2) * H]
                            )
                    xt_t, xq_t, xa_t, _, _ = tiles[pr]
                    ksl = slice(kb * 128, (kb + 1) * 128)
                    s_ab = s_psum.tile([128, 2 * QB], F32, tag="s")
                    nc.tensor.matmul(
                        s_ab[:, 0:QB], xt_t[0:64, ksl], xq_t[0:64, :],
                        start=True, stop=True,
                    )
                    nc.tensor.matmul(
                        s_ab[:, QB : 2 * QB], xt_t[64:128, ksl], xq_t[64:128, :],
                        start=True, stop=True,
                    )
                    p_ab = p_pool.tile([128, 2 * QB], BF16, tag="p")
                    nc.scalar.activation(
                        p_ab[:], s_ab[:], mybir.ActivationFunctionType.Exp,
                        scale=SCALE,
                    )
                    cur = (gs, p_ab)
                if cur is not None:
                    pipe.append(cur)
                prev = pipe.pop(0) if (len(pipe) > SKEW or cur is None) and pipe else None
                if prev is not None:
                    gsp, pp = prev
                    prp, kbp = divmod(gsp, KBLKS)
                    if kbp == 0:
                        ctx_a = ctx_psum.tile([128, QSUB * AUG], F32, tag="ctx",
                                              name=f"ctxa{prp}")
                        ctx_b = ctx_psum.tile([128, QSUB * AUG], F32, tag="ctx",
                                              name=f"ctxb{prp}")
                        tiles[prp][3] = ctx_a
                        tiles[prp][4] = ctx_b
                    _, _, xa_t, ctx_a, ctx_b = tiles[prp]
                    for j, ctx_t in ((0, ctx_a), (1, ctx_b)):
                        asl = slice(j * KBLKS * AUG + kbp * AUG,
                                    j * KBLKS * AUG + (kbp + 1) * AUG)
                        for qs in range(QSUB):
                            # The 4 qs accumulation groups share one psum
                            # bank (= one 2KB zero region): only the tile's
                            # first matmul may set start (start marks the
                            # WHOLE region pending-zero, so later groups'
                            # first writes still overwrite-not-accumulate),
                            # and only the last sets stop.
                            nc.tensor.matmul(
                                ctx_t[:, qs * AUG : (qs + 1) * AUG],
                                pp[:, j * QB + qs * 128 : j * QB + (qs + 1) * 128],
                                xa_t[:, asl],
                                start=(kbp == 0 and qs == 0),
                                stop=(kbp == KBLKS - 1 and qs == QSUB - 1),
                            )
                    g = plan.get(gsp)
                    if g is not None:
                        with tc.tile_wait_until(floor_ms(gsp)):
                            emit_proj_group(g[0], g[1], g[2], prj_psum, "prj")
                    if kbp == KBLKS - 1:
                        emit_drain(prp, ctx_a, ctx_b)
    nc.compile()
    return nc


def _to_bf16(x):
    import ml_dtypes

    return np.asarray(x, np.float32).astype(ml_dtypes.bfloat16)


def shard_inputs(key, W_ctx, b_ctx):
    """Host-side prep of per-core input dicts (bf16 layouts)."""
    Bv = key.shape[0]
    cores_per_batch = NCORES // Bv

    key = np.asarray(key, dtype=np.float32)
    xh = key.reshape(Bv, L, NH, DH)
    # xt: [B, NPAIR, 128, L]; pair p rows 0:64 = head 2p, 64:128 = head 2p+1
    xt_full = np.ascontiguousarray(
        xh.transpose(0, 2, 3, 1).reshape(Bv, NPAIR, 2 * DH, L)
    )
    # xa: [B, NH, 128, KBLKS*AUG] with ones in column kb*AUG+DH, then pair-merged
    xa_full = np.empty((Bv, NH, 128, KBLKS * AUG), dtype=np.float32)
    xa_view = xa_full.reshape(Bv, NH, 128, KBLKS, AUG)
    xa_view[..., DH] = 1.0
    xa_view[..., 0:DH] = xh.reshape(Bv, KBLKS, 128, NH, DH).transpose(0, 3, 2, 1, 4)
    # [B, NPAIR, 2, 128, KA] -> [B, NPAIR, 128, 2*KA]
    ka = KBLKS * AUG
    xa_pair = np.ascontiguousarray(
        xa_full.reshape(Bv, NPAIR, 2, 128, ka).transpose(0, 1, 3, 2, 4)
        .reshape(Bv, NPAIR, 128, 2 * ka)
    )
    wt_host = np.ascontiguousarray(
        np.asarray(W_ctx, np.float32).T.reshape(HC, 128, H).transpose(1, 0, 2)
        .reshape(128, HC * H)
    )
    bias_host = np.asarray(b_ctx, np.float32).reshape(1, H)
    ones_host = np.ones((1, 128), dtype=np.float32)
    ident_host = np.eye(128, dtype=np.float32)

    xt_b = _to_bf16(xt_full)
    xa_b = _to_bf16(xa_pair)
    wt_b = _to_bf16(wt_host)
    bias_b = _to_bf16(bias_host)
    ones_b = _to_bf16(ones_host)
    ident_b = _to_bf16(ident_host)

    in_maps = []
    meta = []
    for c in range(NCORES):
        b = c // cores_per_batch
        qlo = (c % cores_per_batch) * QB
        in_maps.append(
            {
                "xt": xt_b[b],
                "xq": np.ascontiguousarray(xt_b[b][:, :, qlo : qlo + QB]),
                "xa": xa_b[b],
                "wt": wt_b,
                "bias": bias_b,
                "ones": ones_b,
                "ident": ident_b,
            }
        )
        meta.append((b, qlo))
    return in_maps, meta


_NC_CACHE = {}


def kernel(key, W_ctx, b_ctx):
    from concourse.bass_utils import run_bass_kernel_spmd

    key = np.asarray(key, dtype=np.float32)
    if "nc" not in _NC_CACHE:
        _NC_CACHE["nc"] = build_nc()
    nc = _NC_CACHE["nc"]
    in_maps, meta = shard_inputs(key, W_ctx, b_ctx)
    res = run_bass_kernel_spmd(nc, in_maps, list(range(NCORES)))
    outf = np.empty((B, L, H), dtype=np.float32)
    for c, (b, qlo) in enumerate(meta):
        outf[b, qlo : qlo + QB] = res.results[c]["out"]
    return outf


# revision 19
# speedup vs baseline: 1.0418x; 1.0418x over previous
"""Multi-head self-attention (QK^T -> softmax -> ctx -> linear) on 8 TRN2 cores.

Sharding: each core owns one (batch, query-block) shard: batch = core//4,
queries [qlo, qlo+512) with qlo = (core%4)*512. Attention needs all keys of
the core's batch, so keys are replicated per batch; no collectives needed.

Per core (head h, its 512 queries q, all 2048 keys k), all operands bf16:
  S_T[k, q]  = sum_d x[k, hd+d] * x[q, hd+d]            (PE, psum f32)
  P_T[k, q]  = exp(0.125 * S_T[k, q])                   (ACT, psum->sbuf bf16)
  ctx[q, m]  = sum_k P_T[k, q] * xa[k, m]               (PE; P parked as the
               stationary operand so only the 65-wide xa side streams;
               m = 64 dims + ones column -> softmax denominator at m=64)
  chunk[q,i] = ctx[q, d] / ctx[q, 64]                   (DVE tensor_scalar,
               per-partition recip scalar; -> bf16)
  chunkT     = transpose(chunk)                         (DMA xbar transpose)
  out[q, o]  = sum_i chunkT[i, q] * W[o, i] + b[o]      (PE + DVE adds)

The PE cost model charges only streamed output columns (stationary loads are
free), so parking P halves ctx cost vs streaming it; everything else is
orientation-chosen to keep output partitions full.
"""

import sys

for _p in ("/opt/trn_rl_repo", "/root/.axon_site/_ro/trn_rl_repo"):
    if _p not in sys.path:
        sys.path.append(_p)

import numpy as np

import concourse.bacc as bacc
import concourse.bass as bass
import concourse.mybir as mybir
import concourse.tile as tile

F32 = mybir.dt.float32
BF16 = mybir.dt.bfloat16

B, L, H, NH, DH = 2, 2048, 1024, 16, 64
NCORES = 8
QB = 512
KBLKS = L // 128          # 16 key blocks
NPAIR = NH // 2           # 8 head pairs
AUG = DH + 1              # 64 dims + ones column
HC = H // 128             # 8 hidden chunks (one per pair)
OBW = 512                 # proj output column block
SCALE = float(1.0 / np.sqrt(DH))
QSUB = QB // 128          # 4 query subtiles


def build_nc():
    nc = bacc.Bacc("TRN2")
    xt = nc.declare_dram_parameter("xt", [NPAIR, 128, L], BF16, isOutput=False)
    xq = nc.declare_dram_parameter("xq", [NPAIR, 128, QB], BF16, isOutput=False)
    xa = nc.declare_dram_parameter("xa", [NPAIR, 128, 2 * KBLKS * AUG], BF16, isOutput=False)
    wt = nc.declare_dram_parameter("wt", [128, HC * H], BF16, isOutput=False)
    bias = nc.declare_dram_parameter("bias", [1, H], BF16, isOutput=False)
    ones = nc.declare_dram_parameter("ones", [1, 128], BF16, isOutput=False)
    ident = nc.declare_dram_parameter("ident", [128, 128], BF16, isOutput=False)
    out = nc.declare_dram_parameter("out", [QB, H], F32, isOutput=True)

    NSTEP = NPAIR * KBLKS
    SKEW = 2
    # scheduling floor per global step (ns): keeps the tile scheduler from
    # hoisting proj work into much earlier PE positions, where an unmet
    # transpose dep would stall the PE counter that gates the exp stream.
    TSTEP_NS = 1040.0
    TBASE_NS = 4600.0

    def floor_ms(gs):
        return (TBASE_NS + gs * TSTEP_NS) / 1e6

    with tile.TileContext(nc) as tc:
        with (
            tc.tile_pool(name="xt", bufs=2) as xt_pool,
            tc.tile_pool(name="xq", bufs=2) as xq_pool,
            tc.tile_pool(name="xa", bufs=2) as xa_pool,
            tc.tile_pool(name="p", bufs=4) as p_pool,
            tc.tile_pool(name="cq", bufs=8) as cq_pool,
            tc.tile_pool(name="rc", bufs=4) as rc_pool,
            tc.tile_pool(name="consts", bufs=1) as consts,
            tc.tile_pool(name="spsum", bufs=2, space="PSUM") as s_psum,
            tc.tile_pool(name="ctxpsum", bufs=3, space="PSUM") as ctx_psum,
            tc.tile_pool(name="prjpsum", bufs=1, space="PSUM") as prj_psum,
        ):
            ones_t = consts.tile([1, 128], BF16)
            ident_t = consts.tile([128, 128], BF16)
            bias_t = consts.tile([1, H], BF16)
            bias_bc = consts.tile([128, H], F32)

            wt_ts = [
                consts.tile([128, H], BF16, tag=f"wt{c}", name=f"wt{c}")
                for c in range(HC)
            ]
            chunks = [
                consts.tile([128, QB], BF16, tag=f"ch{c}", name=f"ch{c}")
                for c in range(HC)
            ]
            acc = [
                consts.tile([128, H], F32, tag=f"acc{q}", name=f"acc{q}")
                for q in range(QSUB)
            ]

            def pair_dmas(pr, split_xt=False):
                xt_t = xt_pool.tile([128, L], BF16, tag="xt", name=f"xt{pr}")
                xq_t = xq_pool.tile([128, QB], BF16, tag="xq", name=f"xq{pr}")
                nc.sync.dma_start(xq_t[:], xq[pr])
                if split_xt:
                    # first key quarter lands fast so scores can start early
                    nc.sync.dma_start(xt_t[:, 0:512], xt[pr][:, 0:512])
                    nc.sync.dma_start(xt_t[:, 512:L], xt[pr][:, 512:L])
                else:
                    nc.sync.dma_start(xt_t[:], xt[pr])
                xa_t = xa_pool.tile([128, 2 * KBLKS * AUG], BF16, tag="xa", name=f"xa{pr}")
                nc.sync.dma_start(xa_t[:], xa[pr])
                return [xt_t, xq_t, xa_t, None, None]

            def emit_proj_group(clist, qs, ob, pool, tag, eng=None):
                obsl = slice(ob * OBW, (ob + 1) * OBW)
                qsl = slice(qs * 128, (qs + 1) * 128)
                cp = pool.tile([128, 2 * QB] if tag == "s" else [128, OBW],
                               F32, tag=tag, name=f"cp{clist[0]}_{qs}_{ob}")
                for i, c in enumerate(clist):
                    nc.tensor.matmul(
                        cp[:, 0:OBW], chunks[c][:, qsl], wt_ts[c][:, obsl],
                        start=(i == 0), stop=(i == len(clist) - 1),
                    )
                prev = bias_bc if clist[0] == 0 else acc[qs]
                (eng or nc.vector).tensor_add(
                    acc[qs][:, obsl], cp[:, 0:OBW], prev[:, obsl]
                )

            def emit_drain(prp, ctx_a, ctx_b):
                # normalize pair prp's ctx accumulators and transpose into
                # the proj chunk tile; for the final pair, pipeline the
                # remaining projection groups and output DMAs per q-subtile
                last = prp == NPAIR - 1
                rc_a = rc_pool.tile([128, QSUB], F32, tag="rc", name=f"rca{prp}")
                rc_b = rc_pool.tile([128, QSUB], F32, tag="rc", name=f"rcb{prp}")
                with nc.allow_low_precision(reason="softmax denominator recip"):
                    nc.vector.reciprocal(rc_a[:], ctx_a[:, DH : QSUB * AUG : AUG])
                    nc.vector.reciprocal(rc_b[:], ctx_b[:, DH : QSUB * AUG : AUG])
                for qs in range(QSUB):
                    cq = cq_pool.tile([128, 128], BF16, tag="cq", name=f"cq{prp}_{qs}")
                    nc.vector.tensor_scalar_mul(
                        cq[:, 0:DH], ctx_a[:, qs * AUG : qs * AUG + DH],
                        rc_a[:, qs : qs + 1],
                    )
                    nc.vector.tensor_scalar_mul(
                        cq[:, DH : 2 * DH], ctx_b[:, qs * AUG : qs * AUG + DH],
                        rc_b[:, qs : qs + 1],
                    )
                    if not last:
                        nc.sync.dma_start_transpose(
                            chunks[prp][:, qs * 128 : (qs + 1) * 128], cq[:, :]
                        )
                        continue
                    # final pair: PE-transpose (HWDGE is slow to drain) + ACT
                    # copy; fused (5,6,7) groups follow in a second pass so
                    # the DVE adds overlap later q-subtiles' normalizes
                    tp = s_psum.tile([128, 128], BF16, tag="s", name=f"tp{qs}")
                    nc.tensor.transpose(tp[:], cq[:, :], ident_t[:])
                    qsl = slice(qs * 128, (qs + 1) * 128)
                    nc.scalar.copy(chunks[prp][:, qsl], tp[:])
                if last:
                    for qs in range(QSUB):
                        qsl = slice(qs * 128, (qs + 1) * 128)
                        cp = s_psum.tile([128, 2 * QB], F32, tag="s", name=f"tcp{qs}")
                        for ob in range(2):
                            obsl = slice(ob * OBW, (ob + 1) * OBW)
                            for i, c in enumerate((5, 6, 7)):
                                nc.tensor.matmul(
                                    cp[:, obsl], chunks[c][:, qsl], wt_ts[c][:, obsl],
                                    start=(i == 0), stop=(i == 2),
                                )
                        nc.vector.tensor_add(acc[qs][:], cp[:], acc[qs][:])
                        nc.sync.dma_start(out[qsl, :], acc[qs][:])

            # proj emission plan: step -> (chunk list, qs, ob)
            # chunk-pairs (0,1)@pairs2-3, (2,3)@pairs4-5; chunk 4 singles@pair6,
            # (5,6)@pair7, chunk 7 + leftovers in the tail.
            # window steps start ~4 kbs after the newer chunk's drain so the
            # first group never waits on an in-flight transpose
            plan = {}
            for hp, base_pr in ((0, 2), (1, 4)):
                cl = [2 * hp, 2 * hp + 1]
                steps = [base_pr * KBLKS + k for k in (6, 8, 10, 12, 14)] + [
                    (base_pr + 1) * KBLKS + k for k in (4, 8, 12)
                ]
                for gidx, st in enumerate(steps):
                    qs, ob = divmod(gidx, 2)
                    plan[st] = (cl, qs, ob)
            for i, kbw in enumerate((3, 5, 7, 9, 11, 13, 15)):
                plan[6 * KBLKS + kbw] = ([4], i // 2, i % 2)
            plan[7 * KBLKS + 1] = ([4], 3, 1)

            tiles = {}
            pipe = []
            for gs in range(NSTEP + SKEW):
                cur = None
                if gs < NSTEP:
                    pr, kb = divmod(gs, KBLKS)
                    if pr == 0 and kb == 0:
                        warm = consts.tile([1, 512], BF16)
                        nc.vector.memset(warm[:], 1.0)
                        for wi in range(8):
                            wps = prj_psum.tile([128, OBW], F32, tag="prj",
                                                name=f"warm{wi}")
                            nc.tensor.matmul(
                                wps[:], warm[0:1, 0:128], warm[0:1, :],
                                start=True, stop=True,
                            )
                        tiles[0] = pair_dmas(0, split_xt=True)
                        nc.sync.dma_start(wt_ts[0][:], wt[:, 0:H])
                        nc.sync.dma_start(ones_t[:], ones[:])
                        nc.sync.dma_start(bias_t[:], bias[:])
                        nc.sync.dma_start(ident_t[:], ident[:])
                    if pr == 0 and kb == 6:
                        # bias broadcast across partitions via K=1 ones matmul
                        # (uses the proj psum bank, idle until pair 2, so the
                        # late-landing bias DMA never stalls the score slots)
                        for ob in range(H // OBW):
                            obsl = slice(ob * OBW, (ob + 1) * OBW)
                            bps = prj_psum.tile([128, OBW], F32, tag="prj")
                            nc.tensor.matmul(
                                bps[:], ones_t[0:1, :], bias_t[0:1, obsl],
                                start=True, stop=True,
                            )
                            nc.vector.tensor_copy(bias_bc[:, obsl], bps[:])
                    if kb == 2 and pr + 1 < NPAIR:
                        # floor keeps prefetches behind the previous
                        # drain's transposes in the HWDGE queue
                        with tc.tile_wait_until(floor_ms((pr + 1) * KBLKS - 6)):
                            tiles[pr + 1] = pair_dmas(pr + 1)
                    if kb == 4 and pr + 1 < NPAIR:
                        with tc.tile_wait_until(floor_ms(min((pr + 3) * KBLKS - 10, NSTEP))):
                            nc.sync.dma_start(
                                wt_ts[pr + 1][:], wt[:, (pr + 1) * H : (pr + 2) * H]
                            )
                    xt_t, xq_t, xa_t, _, _ = tiles[pr]
                    ksl = slice(kb * 128, (kb + 1) * 128)
                    s_ab = s_psum.tile([128, 2 * QB], F32, tag="s")
                    nc.tensor.matmul(
                        s_ab[:, 0:QB], xt_t[0:64, ksl], xq_t[0:64, :],
                        start=True, stop=True,
                    )
                    nc.tensor.matmul(
                        s_ab[:, QB : 2 * QB], xt_t[64:128, ksl], xq_t[64:128, :],
                        start=True, stop=True,
                    )
                    p_ab = p_pool.tile([128, 2 * QB], BF16, tag="p")
                    nc.scalar.activation(
                        p_ab[:], s_ab[:], mybir.ActivationFunctionType.Exp,
                        scale=SCALE,
                    )
                    cur = (gs, p_ab)
                if cur is not None:
                    pipe.append(cur)
                prev = pipe.pop(0) if (len(pipe) > SKEW or cur is None) and pipe else None
                if prev is not None:
                    gsp, pp = prev
                    prp, kbp = divmod(gsp, KBLKS)
                    if kbp == 0:
                        ctx_a = ctx_psum.tile([128, QSUB * AUG], F32, tag="ctx",
                                              name=f"ctxa{prp}")
                        ctx_b = ctx_psum.tile([128, QSUB * AUG], F32, tag="ctx",
                                              name=f"ctxb{prp}")
                        tiles[prp][3] = ctx_a
                        tiles[prp][4] = ctx_b
                    _, _, xa_t, ctx_a, ctx_b = tiles[prp]
                    for j, ctx_t in ((0, ctx_a), (1, ctx_b)):
                        asl = slice(j * KBLKS * AUG + kbp * AUG,
                                    j * KBLKS * AUG + (kbp + 1) * AUG)
                        for qs in range(QSUB):
                            # The 4 qs accumulation groups share one psum
                            # bank (= one 2KB zero region): only the tile's
                            # first matmul may set start (start marks the
                            # WHOLE region pending-zero, so later groups'
                            # first writes still overwrite-not-accumulate),
                            # and only the last sets stop.
                            nc.tensor.matmul(
                                ctx_t[:, qs * AUG : (qs + 1) * AUG],
                                pp[:, j * QB + qs * 128 : j * QB + (qs + 1) * 128],
                                xa_t[:, asl],
                                start=(kbp == 0 and qs == 0),
                                stop=(kbp == KBLKS - 1 and qs == QSUB - 1),
                            )
                    g = plan.get(gsp)
                    if g is not None:
                        with tc.tile_wait_until(floor_ms(gsp)):
                            emit_proj_group(g[0], g[1], g[2], prj_psum, "prj")
                    if kbp == KBLKS - 1:
                        emit_drain(prp, ctx_a, ctx_b)
    nc.compile()
    return nc


def _to_bf16(x):
    import ml_dtypes

    return np.asarray(x, np.float32).astype(ml_dtypes.bfloat16)


def shard_inputs(key, W_ctx, b_ctx):
    """Host-side prep of per-core input dicts (bf16 layouts)."""
    Bv = key.shape[0]
    cores_per_batch = NCORES // Bv

    key = np.asarray(key, dtype=np.float32)
    xh = key.reshape(Bv, L, NH, DH)
    # xt: [B, NPAIR, 128, L]; pair p rows 0:64 = head 2p, 64:128 = head 2p+1
    xt_full = np.ascontiguousarray(
        xh.transpose(0, 2, 3, 1).reshape(Bv, NPAIR, 2 * DH, L)
    )
    # xa: [B, NH, 128, KBLKS*AUG] with ones in column kb*AUG+DH, then pair-merged
    xa_full = np.empty((Bv, NH, 128, KBLKS * AUG), dtype=np.float32)
    xa_view = xa_full.reshape(Bv, NH, 128, KBLKS, AUG)
    xa_view[..., DH] = 1.0
    xa_view[..., 0:DH] = xh.reshape(Bv, KBLKS, 128, NH, DH).transpose(0, 3, 2, 1, 4)
    # [B, NPAIR, 2, 128, KA] -> [B, NPAIR, 128, 2*KA]
    ka = KBLKS * AUG
    xa_pair = np.ascontiguousarray(
        xa_full.reshape(Bv, NPAIR, 2, 128, ka).transpose(0, 1, 3, 2, 4)
        .reshape(Bv, NPAIR, 128, 2 * ka)
    )
    wt_host = np.ascontiguousarray(
        np.asarray(W_ctx, np.float32).T.reshape(HC, 128, H).transpose(1, 0, 2)
        .reshape(128, HC * H)
    )
    bias_host = np.asarray(b_ctx, np.float32).reshape(1, H)
    ones_host = np.ones((1, 128), dtype=np.float32)
    ident_host = np.eye(128, dtype=np.float32)

    xt_b = _to_bf16(xt_full)
    xa_b = _to_bf16(xa_pair)
    wt_b = _to_bf16(wt_host)
    bias_b = _to_bf16(bias_host)
    ones_b = _to_bf16(ones_host)
    ident_b = _to_bf16(ident_host)

    in_maps = []
    meta = []
    for c in range(NCORES):
        b = c // cores_per_batch
        qlo = (c % cores_per_batch) * QB
        in_maps.append(
            {
                "xt": xt_b[b],
                "xq": np.ascontiguousarray(xt_b[b][:, :, qlo : qlo + QB]),
                "xa": xa_b[b],
                "wt": wt_b,
                "bias": bias_b,
                "ones": ones_b,
                "ident": ident_b,
            }
        )
        meta.append((b, qlo))
    return in_maps, meta


_NC_CACHE = {}


def kernel(key, W_ctx, b_ctx):
    from concourse.bass_utils import run_bass_kernel_spmd

    key = np.asarray(key, dtype=np.float32)
    if "nc" not in _NC_CACHE:
        _NC_CACHE["nc"] = build_nc()
    nc = _NC_CACHE["nc"]
    in_maps, meta = shard_inputs(key, W_ctx, b_ctx)
    res = run_bass_kernel_spmd(nc, in_maps, list(range(NCORES)))
    outf = np.empty((B, L, H), dtype=np.float32)
    for c, (b, qlo) in enumerate(meta):
        outf[b, qlo : qlo + QB] = res.results[c]["out"]
    return outf
